# revision 29
# baseline (speedup 1.0000x reference)
"""Trainium2 Bass kernel for fused sparse-attention block (nn_Attention_790273982568).

Full (unsharded) inputs in, full output out. Internally: tensor-parallel over
heads across 8 NeuronCores — each core owns 4 Q heads + 1 KV head (wqkv rows)
and 512 output columns of wo (rows of wo), with per-head on-device AllGathers
of the attention outputs (overlapped with attention) before the output
projection.
"""

import os
import sys

import numpy as np

for _p in ("/opt/trn_rl_repo", "/root/.axon_site/_ro/trn_rl_repo"):
    if _p not in sys.path and os.path.isdir(_p):
        sys.path.append(_p)

import ml_dtypes  # noqa: E402

import bass_rust as _bass_rust  # noqa: E402
import concourse.bass as bass  # noqa: E402
from concourse import bacc  # noqa: E402
import concourse.mybir as mybir  # noqa: E402
import concourse.tile as tile  # noqa: E402
from concourse.bass import ds, ts  # noqa: E402
from concourse.bass_utils import run_bass_kernel_spmd  # noqa: E402

# Problem shapes (hardcoded per spec)
T = 2048
DIM = 4096
HD = 128
NH = 32
NKV = 8
NCORES = 8
QH = NH // NCORES          # 4 q heads per core
FEAT = (QH + 2) * HD       # 768 qkv features per core
OUTC = DIM // NCORES       # 512 output columns per core
P = 128
NT = T // P                # 16 token tiles
KC = DIM // P              # 32 contraction chunks
QSPAN = 512
NQS = T // QSPAN           # 4 q spans
HALF = HD // 2
EPS = 1e-5
THETA = 10000.0
SCALE = 1.0 / float(np.sqrt(HD))

BF16 = mybir.dt.bfloat16
F32 = mybir.dt.float32
F8 = mybir.dt.float8e4
DR = mybir.MatmulPerfMode.DoubleRow
AX = mybir.AxisListType
ALU = mybir.AluOpType
ACTF = mybir.ActivationFunctionType

# res3 fp8: A@B ~= Ah@Bh + Ah@Bl + Al@Bh with hi/lo e4m3 splits (same scale).
QKV_RES3 = True
OUT_RES3 = True
SX = 16.0      # x scale before split
SW = 256.0     # wqkv/wo scale before split
SAO = 16.0     # ao scale before split (via vaug ones column = 1/SAO)

_PROGRAM_CACHE = {}


def _build_body(nc, aps):
    xT_pair = aps["xT_pair"]        # list of (hi, lo) or [xT] when not res3
    wqkvT_pair = aps["wqkvT_pair"]
    woT_pair = aps["woT_pair"]
    ropeP = aps["ropeP"]
    lnwb = aps["lnwb"]
    masks = aps["masks"]
    ident = aps["ident"]
    ag_in = aps["ag_in"]
    ag_out = aps["ag_out"]       # [QH, NCORES*P, T]
    outT = aps["outT"]
    tc = aps["tc"]

    nx = len(xT_pair)               # 2 when QKV_RES3 else 1
    xdt = F8 if QKV_RES3 else BF16
    aodt = F8 if OUT_RES3 else BF16

    with tc.tile_pool(name="consts", bufs=1) as consts:
        ident_sb = consts.tile([P, P], BF16, tag="ident")
        # (ident load is issued inside the phase-1 warmup stream, after the
        # first w/x calls, so it doesn't delay the first matmul's operands)
        masks_sb = consts.tile([P, 4, QSPAN], BF16, tag="masks")

        # persistent activation strips
        qkT = consts.tile([P, QH + 1, T], BF16, tag="qkT")       # [hd, head, tok]
        vaug = consts.tile([P, NT, HD + 1], BF16, tag="vaug")    # [ktok%, ktile, hd+1]

        prq_ctx = tc.tile_pool(name="prq", bufs=4)
        prq_pool = prq_ctx.__enter__()
        rq_pend = []

        def drain_rq(pool, keep):
            while len(rq_pend) > keep:
                ph, pt, prq = rq_pend.pop(0)
                ptr = pool.tile([P, P], BF16, tag="pt2")
                nc.tensor.transpose(ptr[:], prq, ident_sb[:])
                nc.vector.tensor_copy(qkT[:, ph, ts(pt, P)], ptr[:])

        # ---------------- Phase 1: QKV projection + LN + RoPE ----------------
        with (
            tc.tile_pool(name="wq", bufs=1) as wq_pool,
            tc.tile_pool(name="p1", bufs=3) as p1,
            tc.tile_pool(name="px", bufs=2) as px,
            tc.tile_pool(name="p1s", bufs=4) as p1s,
            tc.tile_pool(name="psum1", bufs=3, space="PSUM") as psum1,
            tc.tile_pool(name="psumT", bufs=2, space="PSUM") as psumT,
        ):
            # x tiles cover 4 token-tiles (512 cols) per group: the 512B
            # contiguous runs hit full DMA bandwidth (sub-512B runs pay 2x),
            # and one dma_start per stream keeps the shared HWDGE dispatcher
            # (625ns/call) out of the way.
            def load_xg(g):
                tiles = []
                for i in range(nx):
                    tl = px.tile([P, KC, 4 * P], xdt, tag=f"xt{i}",
                                 name=f"xt{i}_g{g}")
                    for hf in range(2):
                        (nc.sync if (i + hf) % 2 == 0 else nc.scalar).dma_start(
                            tl[:, ds(16 * hf, 16), :],
                            xT_pair[i][ds(16 * hf * P, 16 * P),
                                       ds(g * 4 * P, 4 * P)].rearrange(
                                "(k p) c -> p k c", p=P
                            ),
                        )
                    tiles.append(tl)
                return tiles

            xt0 = [
                px.tile([P, KC, 4 * P], xdt, tag=f"xt{i}", name=f"xt{i}_g0")
                for i in range(nx)
            ]
            wqkvT_sb = [
                wq_pool.tile([P, KC, FEAT], xdt, tag=f"wqkvT{i}",
                             name=f"wqkvT{i}")
                for i in range(nx)
            ]
            # warmup streaming: w in 4-chunk calls, x in quarter calls,
            # interleaved so delivery tracks the kp-major consumption of
            # tiles 0-2. Each dma_start costs ~625ns on the shared HWDGE
            # dispatcher, so w stays at 4-chunk granularity (transfer >
            # dispatch) while x is coarse.
            rope_sb = wq_pool.tile([P, NT, 2, HALF], F32, tag="rope")

            def _wload(i, k0, klen, e):
                e.dma_start(
                    wqkvT_sb[i][:, ds(k0, klen), :],
                    wqkvT_pair[i][ds(k0 * P, klen * P), :].rearrange(
                        "(k p) f -> p k f", p=P
                    ),
                )

            def _xload(i, k0, klen, e):
                e.dma_start(
                    xt0[i][:, ds(k0, klen), :],
                    xT_pair[i][ds(k0 * P, klen * P), ds(0, 4 * P)].rearrange(
                        "(k p) c -> p k c", p=P
                    ),
                )

            # single queue: transfers execute in strict emission order, so
            # delivery exactly tracks the matmul consumption order below
            q = nc.sync
            _wload(0, 0, 2, q)
            _xload(0, 0, 2, q)
            _wload(1, 0, 2, q)
            _wload(0, 2, 2, q)
            _xload(0, 2, 6, q)
            _wload(1, 2, 2, q)
            _wload(0, 4, 4, q)
            _wload(1, 4, 4, q)
            _xload(1, 0, 8, q)
            _wload(0, 8, 4, q)
            _xload(0, 8, 8, q)
            _wload(1, 8, 4, q)
            _wload(0, 12, 4, q)
            _wload(1, 12, 4, q)
            _xload(1, 8, 8, q)
            _wload(0, 16, 4, q)
            _xload(0, 16, 8, q)
            _wload(1, 16, 4, q)
            q.dma_start(rope_sb[:, 0:8], ropeP[:, 0:8, :, :])
            _wload(0, 20, 4, q)
            _wload(1, 20, 4, q)
            _xload(1, 16, 8, q)
            _wload(0, 24, 4, q)
            _wload(1, 24, 4, q)
            q.dma_start(ident_sb[:], ident[:, :])
            _xload(0, 24, 8, q)
            _xload(1, 24, 8, q)
            _wload(0, 28, 4, q)
            _wload(1, 28, 4, q)
            xt_cache = {0: xt0}
            if not aps.get("ln_trivial"):
                wb_sb = wq_pool.tile([P, 2, 2, HD], F32, tag="wb")
                nc.sync.dma_start(wb_sb[:], lnwb[:, :, :, :])

            eps_sb = wq_pool.tile([P, 1], F32, tag="eps")
            nc.vector.memset(eps_sb[:], EPS)

            assert QKV_RES3
            # 3-term residual fp8: (xh+xl)@(wh+wl) minus xl@wl, via
            # DoubleRow over adjacent k-chunk pairs. All terms share
            # the SX*SW scale; descaled at PSUM evacuation.
            terms = [(0, 0), (1, 0), (0, 1)]
            NP = KC // 2

            def qkv_mms(pq, xt_tiles, sub, ti_sel):
                for idx, ti in enumerate(ti_sel):
                    xi, wi = terms[ti]
                    for kp in range(NP):
                        st = kp == 0 and idx == 0
                        sp = kp == NP - 1 and idx == len(ti_sel) - 1
                        lhsT = xt_tiles[xi][:, ds(2 * kp, 2), ds(sub * P, P)]
                        nc.tensor.matmul(
                            pq[:, 0:512], lhsT,
                            wqkvT_sb[wi][:, ds(2 * kp, 2), 0:512],
                            start=st, stop=sp, perf_mode=DR,
                        )
                        nc.tensor.matmul(
                            pq[:, 512:FEAT], lhsT,
                            wqkvT_sb[wi][:, ds(2 * kp, 2), 512:FEAT],
                            start=st, stop=sp, perf_mode=DR,
                        )

            def post_tile(t, pq):
                descale = 1.0 / (SX * SW)
                # transposes of earlier ropes after t's matmuls: their
                # psum-bank evacs completed during t-1's LN chain, so PE
                # never stalls (a stall would reset the p-state ramp)
                drain_rq(psumT, 5)
                # v slice straight to vaug (round f32->bf16 exactly once);
                # evacuations on ACT (DVE is the phase-1 bottleneck)
                nc.scalar.activation(
                    vaug[:, t, 0:HD], pq[:, 640:FEAT], ACTF.Copy, scale=descale
                )
                nc.vector.memset(
                    vaug[:, t, HD : HD + 1], (1.0 / SAO) if OUT_RES3 else 1.0
                )
                # q/k slices as bf16 (match reference's bf16 xqkv)
                xq = p1.tile([P, 5 * HD], BF16, tag="xq")
                nc.scalar.activation(
                    xq[:], pq[:, 0 : 5 * HD], ACTF.Copy, scale=descale
                )

                # ---- batched LN stats over all 5 heads ----
                xq5 = xq.rearrange("p (h d) -> p h d", h=5)
                s1 = p1s.tile([P, 5, 1], F32, tag="s1")
                nc.vector.reduce_sum(s1[:], xq5, axis=AX.X)
                ssq = p1s.tile([P, 5, 1], F32, tag="ssq")
                for h in range(5):
                    sqs = p1s.tile([P, HD], F32, tag="sqs")
                    if t >= NT - 2:
                        # last tiles: square+reduce fused on DVE so the ACT
                        # tail doesn't delay the first phase-2 exps (the ACT
                        # queue is strict FIFO)
                        nc.vector.tensor_tensor_reduce(
                            sqs[:], xq[:, ds(h * HD, HD)], xq[:, ds(h * HD, HD)],
                            1.0, 0.0, op0=ALU.mult, op1=ALU.add,
                            accum_out=ssq[:, h, :],
                        )
                    else:
                        nc.scalar.activation(
                            sqs[:], xq[:, ds(h * HD, HD)], ACTF.Square,
                            accum_out=ssq[:, h, :],
                        )
                negmu = p1s.tile([P, 5, 1], F32, tag="negmu")
                nc.vector.tensor_scalar_mul(negmu[:], s1[:], -1.0 / HD)
                mu2 = p1s.tile([P, 5, 1], F32, tag="mu2")
                nc.vector.tensor_mul(mu2[:], negmu[:], negmu[:])
                var = p1s.tile([P, 5, 1], F32, tag="var")
                nc.vector.scalar_tensor_tensor(
                    var[:], ssq[:], 1.0 / HD, mu2[:],
                    op0=ALU.mult, op1=ALU.subtract,
                )
                std = p1s.tile([P, 5, 1], F32, tag="std")
                _si = nc.scalar.activation(std[:], var[:], ACTF.Sqrt, bias=eps_sb[:])
                aps["last_sqrt"] = _si.ins
                rstd = p1s.tile([P, 5, 1], F32, tag="rstd")
                nc.vector.reciprocal(rstd[:], std[:])
                nbias = p1s.tile([P, 5, 1], F32, tag="nbias")
                nc.vector.tensor_mul(nbias[:], negmu[:], rstd[:])
                # ---- per-head center+scale (per-partition scalars) ----
                xn5 = p1.tile([P, 5, HD], F32, tag="xn5")
                for h in range(5):
                    nc.vector.tensor_scalar(
                        xn5[:, h, :], xq[:, ds(h * HD, HD)],
                        rstd[:, h, :], nbias[:, h, :],
                        op0=ALU.mult, op1=ALU.add,
                    )
                if not aps.get("ln_trivial"):
                    # y = xn * w + b; q heads share w/b, k head has its own
                    for qk, h0, nh in ((0, 0, QH), (1, QH, 1)):
                        wv = wb_sb[:, qk, 0, :].unsqueeze(1).broadcast_to(
                            [P, nh, HD]
                        )
                        bv = wb_sb[:, qk, 1, :].unsqueeze(1).broadcast_to(
                            [P, nh, HD]
                        )
                        nc.vector.tensor_mul(
                            xn5[:, ds(h0, nh), :], xn5[:, ds(h0, nh), :], wv
                        )
                        nc.vector.tensor_add(
                            xn5[:, ds(h0, nh), :], xn5[:, ds(h0, nh), :], bv
                        )
                # ---- batched rope over all 5 heads ----
                cosb = rope_sb[:, t, 0, :].unsqueeze(1).broadcast_to(
                    [P, 5, HALF]
                )
                sinb = rope_sb[:, t, 1, :].unsqueeze(1).broadcast_to(
                    [P, 5, HALF]
                )
                xr = xn5.rearrange("p h (f two) -> p h two f", two=2)
                xe = xr[:, :, 0, :]
                xo = xr[:, :, 1, :]
                ta = p1.tile([P, 5, HALF], F32, tag="ta")
                tb = p1.tile([P, 5, HALF], F32, tag="tb")
                rq5 = prq_pool.tile([P, 5, HD], BF16, tag="rq", name=f"rq_{t}")
                rqr = rq5.rearrange("p h (f two) -> p h two f", two=2)
                nc.vector.tensor_mul(ta[:], xe, cosb)
                nc.vector.tensor_mul(tb[:], xo, sinb)
                nc.vector.tensor_sub(rqr[:, :, 0, :], ta[:], tb[:])
                nc.vector.tensor_mul(ta[:], xe, sinb)
                nc.vector.tensor_mul(tb[:], xo, cosb)
                nc.vector.tensor_add(rqr[:, :, 1, :], ta[:], tb[:])
                for h in range(5):
                    rq_pend.append((h, t, rq5[:, h, :]))

            xt_cache[1] = load_xg(1)
            # tiles 0-2: emission follows the warmup stream's landing order:
            # hi@hi + hi@lo per 2-kp w-group (xh lands early, wl right after
            # wh), with the lo-x term as two late passes once each xl
            # quarter has landed. Keeps PE paced right behind the DMA.
            pqs = [
                psum1.tile([P, FEAT], F32, tag="pqkv", name=f"pq_{t}")
                for t in range(3)
            ]

            def _wmm(t, kp, xi, wi, st=False, sp=False):
                lhsT = xt0[xi][:, ds(2 * kp, 2), ds(t * P, P)]
                nc.tensor.matmul(
                    pqs[t][:, 0:512], lhsT,
                    wqkvT_sb[wi][:, ds(2 * kp, 2), 0:512],
                    start=st, stop=sp, perf_mode=DR,
                )
                nc.tensor.matmul(
                    pqs[t][:, 512:FEAT], lhsT,
                    wqkvT_sb[wi][:, ds(2 * kp, 2), 512:FEAT],
                    start=st, stop=sp, perf_mode=DR,
                )

            # (10)-passes sit where their xl quarter lands; wg7 goes last
            # (its w group is the final delivery) and so carries the stop
            xl_pass = {1: range(0, 4), 3: range(4, 8), 5: range(8, 12),
                       6: range(12, 16)}
            for wg in range(8):
                for kp in (2 * wg, 2 * wg + 1):
                    for t in range(3):
                        _wmm(t, kp, 0, 0, st=(kp == 0))
                        _wmm(t, kp, 0, 1, sp=(wg == 7 and kp == NP - 1))
                for kp in xl_pass.get(wg, ()):
                    for t in range(3):
                        _wmm(t, kp, 1, 0)
            for t in range(3):
                post_tile(t, pqs[t])
            xt_tiles = xt0
            for t in range(3, NT):
                if t % 4 == 0 and t > 0:
                    xt_tiles = xt_cache.pop(t // 4)
                    ng = t // 4 + 1
                    if ng < NT // 4:
                        xt_cache[ng] = load_xg(ng)
                if t == 4:
                    # second half of the rope table isn't read until tile 8;
                    # loading it here keeps 256KB out of the warmup window
                    nc.sync.dma_start(rope_sb[:, 8:NT], ropeP[:, 8:NT, :, :])
                if t == 6:
                    # masks aren't read until the first diagonal pair in
                    # phase 2; load them once the warmup DMA crunch is over
                    nc.scalar.dma_start(masks_sb[:], masks[:, :, :])
                if t == NT - 1:
                    # rotate the ring so t15 lands on a slot whose bank the
                    # first phase-2 scores psum won't collide with (its evac
                    # would otherwise gate the first scores matmul)
                    psum1.tile([P, FEAT], F32, tag="pqkv", name="pq_dummy")
                pq = psum1.tile([P, FEAT], F32, tag="pqkv", name=f"pq_{t}")
                # (0,0),(0,1) before (1,0): the lo-x stream of a fresh group
                # lands last, so the xl-dependent term goes last
                qkv_mms(pq, xt_tiles, t % 4, [0, 2, 1])
                post_tile(t, pq)

        # ---------------- Phase 2: attention (+ per-head AllGather) ----------
        with (
            tc.tile_pool(name="w3", bufs=1) as w3,
            tc.tile_pool(name="p3", bufs=12) as p3,
            tc.tile_pool(name="p3o", bufs=3) as p3o,
            tc.tile_pool(name="paoT", bufs=2) as paoT,
        ):
            # prefetch wo weights while attention runs (4 batched dmas keep
            # the shared HWDGE dispatcher free for the ao gathers)
            assert OUT_RES3
            nw = len(woT_pair)
            wodt = F8 if OUT_RES3 else BF16
            woT_sb = [
                w3.tile([P, KC, OUTC], wodt, tag=f"woT{i}", name=f"woT{i}")
                for i in range(nw)
            ]
            for i in range(nw):
                for hf in range(2):
                    (nc.sync if hf == 0 else nc.scalar).dma_start(
                        woT_sb[i][:, ds(16 * hf, 16), :],
                        woT_pair[i][ds(16 * hf * P, 16 * P), :].rearrange(
                            "(k p) f -> p k f", p=P
                        ),
                    )

            # gathered ao tiles: one [P, r2, c, T//2] tile per (half, head,
            # stream); rhs DR pairs slice the rank-pair dim c.
            ao_sb = [[None] * QH, [None] * QH]
            with (
                tc.tile_pool(name="p2", bufs=2) as p2,
                tc.tile_pool(name="p2s", bufs=4) as p2s,
                tc.tile_pool(name="paob", bufs=12) as paob,
                tc.tile_pool(name="psum_t2", bufs=2, space="PSUM") as psum_t2,
                tc.tile_pool(name="psum_o", bufs=2, space="PSUM") as psum_o_pool,
                tc.tile_pool(name="psum_s", bufs=2, space="PSUM") as psum_s_pool,
            ):
                def emit_scores(h, j):
                    nkb = 4 * (j + 1)
                    attn = p2.tile([P, NT, QSPAN], BF16, tag="attn",
                                   name=f"attn_{h}_{j}")
                    for ip in range(nkb // 2):
                        i = 2 * ip
                        r = i - 4 * j
                        # the r=2 diagonal pair is only ever read by PV for
                        # q >= 256 within the span; skip the dead half
                        q0 = 256 if r == 2 else 0
                        qw = QSPAN - q0
                        ps = psum_s_pool.tile([P, 2, QSPAN], F32, tag="ps")
                        for u in range(2):
                            nc.tensor.matmul(
                                ps[:, u, ds(q0, qw)],
                                qkT[:, QH, ts(i + u, P)],
                                qkT[:, h, ds(j * QSPAN + q0, qw)],
                                start=True, stop=True,
                            )
                        # one exp over both blocks (amortize ACT fixed cost)
                        _ei = nc.scalar.activation(
                            attn[:, i : i + 2, ds(q0, qw)],
                            ps[:, :, ds(q0, qw)], ACTF.Exp, scale=SCALE
                        )
                        if h == 0 and aps.get("last_sqrt") is not None:
                            # keep early exps ordered after the last LN sqrt so
                            # the scheduler can't thrash the ACT table set
                            _bass_rust.add_dep_helper(
                                _ei.ins, aps["last_sqrt"], sync=True,
                                reason="ACT table-set ordering",
                            )
                        if r >= 0:
                            # diagonal pair: one masking mul over both blocks
                            nc.vector.tensor_mul(
                                attn[:, i : i + 2, ds(q0, qw)],
                                attn[:, i : i + 2, ds(q0, qw)],
                                masks_sb[:, r : r + 2, ds(q0, qw)],
                            )
                    return attn

                def emit_pv_mm(h, j, attn):
                    aobs = []
                    for q4 in range(4):
                        qb = 4 * j + q4
                        po = psum_o_pool.tile([P, HD + 1], F32, tag="po")
                        for i in range(qb + 1):
                            nc.tensor.matmul(
                                po[:],
                                attn[:, i, ts(q4, P)],
                                vaug[:, i, :],
                                start=(i == 0), stop=(i == qb),
                            )
                        recip = p2s.tile([P, 1], F32, tag="recip")
                        nc.vector.reciprocal(recip[:], po[:, HD : HD + 1])
                        # with OUT_RES3 the ones column is 1/SAO so recip =
                        # SAO/d and aob = SAO*ao (bf16); split happens at the
                        # post-transpose PSUM evacuation
                        aob = paob.tile([P, HD], BF16, tag="aob",
                                        name=f"aob_{h}_{qb}")
                        nc.vector.tensor_scalar_mul(
                            aob[:], po[:, 0:HD], recip[:]
                        )
                        aobs.append(aob)
                    return aobs

                def emit_tr(h, j, aobs, aoTh):
                    for q4 in range(4):
                        qb = 4 * j + q4
                        pt2 = psum_t2.tile([P, P], BF16, tag="pt2")
                        nc.tensor.transpose(pt2[:], aobs[q4][:], ident_sb[:])
                        if OUT_RES3:
                            # evacuate as same-scale e4m3 hi + lo residual
                            nc.vector.tensor_copy(aoTh[0][:, ts(qb, P)], pt2[:])
                            nc.vector.tensor_sub(
                                aoTh[1][:, ts(qb, P)], pt2[:],
                                aoTh[0][:, ts(qb, P)],
                            )
                        else:
                            nc.vector.tensor_copy(aoTh[0][:, ts(qb, P)], pt2[:])
                    if j == 1 or j == NQS - 1:
                        th = 0 if j == 1 else 1
                        for i in range(nw):
                            nc.sync.dma_start(
                                ag_in[h, th, ds(i * HD, HD), :],
                                aoTh[i][:, ds(th * (T // 2), T // 2)],
                            )
                        _post_half(h, th)

                def load_ao_big(h, th, engs=(nc.sync, nc.scalar)):
                    # one dma per stream for the whole gathered head-half:
                    # [P, r2(rank-pair group), c(rank within pair), T//2]
                    tiles = []
                    src = ag_out[h, th].rearrange(
                        "(r2 c i p) t -> p r2 c i t", r2=NCORES // 2, c=2, i=nw
                    )
                    for i in range(nw):
                        a = p3.tile([P, NCORES // 2, 2, T // 2], aodt,
                                    tag="ao", name=f"ao_{th}_{h}_{i}")
                        engs[i % 2].dma_start(a[:], src[:, :, :, i, :])
                        tiles.append(a)
                    return tiles

                def _post_half(h, th):
                    rows = nw * HD
                    if aps.get("no_collective"):
                        # loopback stand-in for the AllGather: one dma with a
                        # 0-stride (broadcast) source writing all 8 rank slots
                        nc.sync.dma_start(
                            ag_out[h, th].rearrange(
                                "(r p) t -> r p t", r=NCORES
                            ),
                            ag_in[h, th].unsqueeze(0).broadcast_to(
                                [NCORES, rows, T // 2]
                            ),
                        )
                    else:
                        nc.gpsimd.collective_compute(
                            "AllGather",
                            ALU.bypass,
                            replica_groups=[list(range(NCORES))],
                            ins=[ag_in[h, th]],
                            outs=[ag_out[h, th]],
                        )
                    if th == 0:
                        # th=0 tiles feed phase 3's first half directly.
                        # Ring-slot order matters: the first 8 "ao" slots
                        # must all be th=0 tiles so the late th=1 h2/h3
                        # loads land on th=0 slots (free once the th=0
                        # groups finish) — so th=1 h0/h1 prefetch is
                        # deferred until after th=0 h3 allocates.
                        ao_sb[0][h] = load_ao_big(h, 0)
                        if h == QH - 1:
                            ao_sb[1][0] = load_ao_big(0, 1, (nc.scalar, nc.sync))
                            ao_sb[1][1] = load_ao_big(1, 1, (nc.scalar, nc.sync))

                # software pipeline: scores(j) | pv(j-2) | transpose(j-3)
                from collections import deque

                pv_q = deque()   # (h, j, attn)
                tr_q = deque()   # (h, j, aobs, aoTh)
                aoThs = {}
                spans = [(h, j) for h in range(QH) for j in range(NQS)]

                def step_pv():
                    ph, pj, pattn = pv_q.popleft()
                    tr_q.append((ph, pj, emit_pv_mm(ph, pj, pattn), aoThs[ph]))

                def step_tr():
                    emit_tr(*tr_q.popleft())

                for si, (h, j) in enumerate(spans):
                    if j == 0:
                        aoThs[h] = [
                            paoT.tile([P, T], aodt, tag=f"aoTh{i}",
                                      name=f"aoT{i}_{h}")
                            for i in range(nw)
                        ]
                    attn = emit_scores(h, j)
                    if si == 2:
                        # the last two tiles' q/k transposes, deferred past
                        # phase-1 end: by now their rope inputs are long done
                        # and only span (0,3)+ needs them
                        drain_rq(psum_t2, 0)
                    pv_q.append((h, j, attn))
                    if len(pv_q) > 1:
                        step_pv()
                    if len(tr_q) > 1:
                        step_tr()
                while pv_q:
                    step_pv()
                    while len(tr_q) > 1:
                        step_tr()
                while tr_q:
                    step_tr()

            # ---------------- Phase 3: output projection ----------------
            out_descale = 1.0 / (SAO * SW)
            with tc.tile_pool(name="psum3", bufs=8, space="PSUM") as psum3:
                def p3_group(th, cbg, ccs, s2s=(0, 1)):
                    pos = {
                        (cc, s2): psum3.tile(
                            [P, 512], F32, tag="po3",
                            name=f"po3_{th}_{cbg}_{cc}_{s2}")
                        for cc in ccs for s2 in s2s
                    }
                    for kp in range(KC // 2):
                        h, r2 = divmod(kp, NCORES // 2)
                        if ao_sb[th][h] is None:
                            ao_sb[th][h] = load_ao_big(h, th)
                        ah, al = ao_sb[th][h]
                        for cc in ccs:
                            cb = cbg * 2 + cc
                            for s2 in s2s:
                                po3 = pos[(cc, s2)]
                                wh = woT_sb[0][:, ds(2 * kp, 2), ts(cb, P)]
                                wl = woT_sb[1][:, ds(2 * kp, 2), ts(cb, P)]
                                st = kp == 0
                                sp = kp == KC // 2 - 1
                                rh = ah[:, r2, :, ds(s2 * 512, 512)]
                                rl = al[:, r2, :, ds(s2 * 512, 512)]
                                nc.tensor.matmul(
                                    po3[:], wh, rh,
                                    start=st, stop=False, perf_mode=DR,
                                )
                                nc.tensor.matmul(
                                    po3[:], wl, rh,
                                    start=False, stop=False, perf_mode=DR,
                                )
                                nc.tensor.matmul(
                                    po3[:], wh, rl,
                                    start=False, stop=sp, perf_mode=DR,
                                )
                    for cc in ccs:
                        cb = cbg * 2 + cc
                        for s2 in s2s:
                            ob = p3o.tile(
                                [P, 512], BF16, tag="ob",
                                name=f"ob_{th}_{cb}_{s2}"
                            )
                            if s2 == 0:
                                nc.vector.tensor_scalar_mul(
                                    ob[:], pos[(cc, s2)][:], out_descale
                                )
                            else:
                                nc.scalar.activation(
                                    ob[:], pos[(cc, s2)][:],
                                    ACTF.Copy, scale=out_descale,
                                )
                            nc.sync.dma_start(
                                outT[ts(cb, P),
                                     ds(th * (T // 2) + s2 * 512, 512)],
                                ob[:],
                            )

                for th in range(2):  # token halves
                    if th == 1:
                        # h2/h3 second-half tiles: emit the loads up front
                        # (their ring slots free as the th=0 readers finish)
                        for h in (2, 3):
                            if ao_sb[1][h] is None:
                                ao_sb[1][h] = load_ao_big(
                                    h, 1, (nc.scalar, nc.sync)
                                )
                    for cbg in range(2):
                        if th == 1 and cbg == 1:
                            # split the final group progressively finer so
                            # each drain overlaps the next piece's matmuls
                            p3_group(th, cbg, (0,))
                            p3_group(th, cbg, (1,), (0,))
                            p3_group(th, cbg, (1,), (1,))
                        else:
                            p3_group(th, cbg, (0, 1))

        prq_ctx.__exit__(None, None, None)


def _build_program(no_collective=False, reps=1, ln_trivial=True):
    nc = bacc.Bacc(
        "TRN2",
        target_bir_lowering=False,
        debug=False,
        enable_asserts=True,
        num_devices=1 if no_collective else NCORES,
    )
    xdt = F8 if QKV_RES3 else BF16
    aodt = F8 if OUT_RES3 else BF16
    nx = 2 if QKV_RES3 else 1
    nw = 2 if OUT_RES3 else 1
    aps = {
        "xT_pair": [
            nc.dram_tensor(f"xT{i}", [DIM, T], xdt, kind="ExternalInput").ap()
            for i in range(nx)
        ],
        "wqkvT_pair": [
            nc.dram_tensor(
                f"wqkvT{i}", [DIM, FEAT], xdt, kind="ExternalInput"
            ).ap()
            for i in range(nx)
        ],
        "woT_pair": [
            nc.dram_tensor(
                f"woT{i}", [NH * HD, OUTC], aodt, kind="ExternalInput"
            ).ap()
            for i in range(nw)
        ],
        "ropeP": nc.dram_tensor(
            "ropeP", [P, NT, 2, HALF], F32, kind="ExternalInput"
        ).ap(),
        "lnwb": nc.dram_tensor("lnwb", [P, 2, 2, HD], F32, kind="ExternalInput").ap(),
        "masks": nc.dram_tensor("masks", [P, 4, QSPAN], BF16, kind="ExternalInput").ap(),
        "ident": nc.dram_tensor("ident", [P, P], BF16, kind="ExternalInput").ap(),
        "ag_in": nc.dram_tensor("ag_in", [QH, 2, nw * HD, T // 2], aodt).ap(),
        "ag_out": nc.dram_tensor(
            "ag_out", [QH, 2, NCORES * nw * HD, T // 2], aodt,
            addr_space="Shared"
        ).ap(),
        "outT": nc.dram_tensor("outT", [OUTC, T], BF16, kind="ExternalOutput").ap(),
    }
    aps["no_collective"] = no_collective
    aps["ln_trivial"] = ln_trivial
    with tile.TileContext(nc) as tc:
        aps["tc"] = tc
        for _rep in range(reps):
            _build_body(nc, aps)
    nc.compile()
    return nc


def get_program(ln_trivial=True):
    key = ("nc", ln_trivial)
    if key not in _PROGRAM_CACHE:
        _PROGRAM_CACHE[key] = _build_program(ln_trivial=ln_trivial)
    return _PROGRAM_CACHE[key]


def _rope_tables():
    """cos/sin tables computed exactly like the reference (jax fp32 on cpu)."""
    try:
        import jax

        cpu = jax.devices("cpu")[0]
        with jax.default_device(cpu):
            import jax.numpy as jnp

            inv_freq = 1.0 / (
                THETA ** (jnp.arange(HALF, dtype=jnp.float32) * 2.0 / HD)
            )
            pos = jnp.arange(T, dtype=jnp.float32)
            ang = pos[:, None] * inv_freq[None, :]
            cos = np.asarray(jnp.cos(ang), dtype=np.float32)
            sin = np.asarray(jnp.sin(ang), dtype=np.float32)
    except Exception:
        inv_freq = (
            1.0 / (THETA ** (np.arange(HALF, dtype=np.float32) * 2.0 / HD))
        ).astype(np.float32)
        ang = np.arange(T, dtype=np.float32)[:, None] * inv_freq[None, :]
        cos = np.cos(ang).astype(np.float32)
        sin = np.sin(ang).astype(np.float32)
    return cos, sin


def _make_const_inputs(q_ln_w, q_ln_b, k_ln_w, k_ln_b):
    cos, sin = _rope_tables()  # [T, HALF] f32
    ropeP = np.zeros((P, NT, 2, HALF), np.float32)
    ropeP[:, :, 0] = cos.reshape(NT, P, HALF).transpose(1, 0, 2)
    ropeP[:, :, 1] = sin.reshape(NT, P, HALF).transpose(1, 0, 2)

    lnwb = np.zeros((P, 2, 2, HD), np.float32)
    lnwb[:, 0, 0] = np.asarray(q_ln_w, np.float32)[None, :]
    lnwb[:, 0, 1] = np.asarray(q_ln_b, np.float32)[None, :]
    lnwb[:, 1, 0] = np.asarray(k_ln_w, np.float32)[None, :]
    lnwb[:, 1, 1] = np.asarray(k_ln_b, np.float32)[None, :]

    f = np.arange(QSPAN)[None, None, :]
    r = np.arange(4)[None, :, None]
    p = np.arange(P)[:, None, None]
    masks = (f >= 128 * r + p).astype(ml_dtypes.bfloat16)  # [P, 4, QSPAN]
    ident = np.eye(P, dtype=ml_dtypes.bfloat16)
    return ropeP, lnwb, masks, ident


# phase-3 lhsT rows are ordered (h, r, d) = head-of-rank h, rank r; the ao
# feature order is (global head g = 4r+h, d). Permute woT rows to match.
_WOT_PERM = np.empty(NH * HD, np.int64)
for _h in range(QH):
    for _r in range(NCORES):
        _j = (_h * NCORES + _r) * HD
        _g = (4 * _r + _h) * HD
        _WOT_PERM[_j : _j + HD] = np.arange(_g, _g + HD)


def _split8(a, scale):
    """a*scale ~= hi + lo, both e4m3 at the same scale."""
    a32 = np.asarray(a, dtype=np.float32) * scale
    hi = a32.astype(ml_dtypes.float8_e4m3)
    lo = (a32 - hi.astype(np.float32)).astype(ml_dtypes.float8_e4m3)
    return np.ascontiguousarray(hi), np.ascontiguousarray(lo)


def make_in_maps(inputs):
    x = np.asarray(inputs["x"], dtype=ml_dtypes.bfloat16)
    wqkv = np.asarray(inputs["wqkv"], dtype=ml_dtypes.bfloat16)
    wo = np.asarray(inputs["wo"], dtype=ml_dtypes.bfloat16)
    q_ln_w = np.asarray(inputs["q_ln_w"], np.float32)
    q_ln_b = np.asarray(inputs["q_ln_b"], np.float32)
    k_ln_w = np.asarray(inputs["k_ln_w"], np.float32)
    k_ln_b = np.asarray(inputs["k_ln_b"], np.float32)

    ropeP, lnwb, masks, ident = _make_const_inputs(q_ln_w, q_ln_b, k_ln_w, k_ln_b)
    xT = np.ascontiguousarray(x.T)
    if QKV_RES3:
        xT_hi, xT_lo = _split8(xT, SX)

    in_maps = []
    for c in range(NCORES):
        qrows = wqkv[c * QH * HD : (c + 1) * QH * HD]
        krows = wqkv[NH * HD + c * HD : NH * HD + (c + 1) * HD]
        vrows = wqkv[(NH + NKV) * HD + c * HD : (NH + NKV) * HD + (c + 1) * HD]
        wqkvT_c = np.ascontiguousarray(
            np.concatenate([qrows, krows, vrows], axis=0).T
        )
        woT_c = np.ascontiguousarray(
            wo[c * OUTC : (c + 1) * OUTC, :].T[_WOT_PERM, :]
        )
        m = {
            "ropeP": ropeP,
            "lnwb": lnwb,
            "masks": masks,
            "ident": ident,
        }
        if QKV_RES3:
            m["xT0"], m["xT1"] = xT_hi, xT_lo
            m["wqkvT0"], m["wqkvT1"] = _split8(wqkvT_c, SW)
        else:
            m["xT0"] = xT
            m["wqkvT0"] = wqkvT_c
        if OUT_RES3:
            m["woT0"], m["woT1"] = _split8(woT_c, SW)
        else:
            m["woT0"] = woT_c
        in_maps.append(m)
    return in_maps


def kernel(**inputs):
    lt = all(
        bool(np.all(np.asarray(inputs[n], np.float32) == v))
        for n, v in (("q_ln_w", 1.0), ("k_ln_w", 1.0),
                     ("q_ln_b", 0.0), ("k_ln_b", 0.0))
    )
    nc = get_program(ln_trivial=lt)
    in_maps = make_in_maps(inputs)
    res = run_bass_kernel_spmd(nc, in_maps, list(range(NCORES)))
    outT_full = np.concatenate(
        [np.asarray(res.results[c]["outT"]) for c in range(NCORES)], axis=0
    )
    return np.ascontiguousarray(outT_full.T).astype(ml_dtypes.bfloat16)


if __name__ == "__main__":
    nc = get_program()
    print("program built ok")

